# revision 5
# baseline (speedup 1.0000x reference)
# Multi-head attention (N=4, L=2048, D=1024, H=16, DK=64) on 8 NeuronCores.
#
# Sharding: pure data-parallel over (batch n, q-half) -> 8 shards, no
# collectives. Each core gets Q rows [n, qh*1024:(qh+1)*1024], full K/V of its
# batch, and the matching mask rows.
#
# Per-core pipeline (all layouts chosen so the contraction dim sits on SBUF
# partitions and softmax row-sums come out of the PE for free):
#   QiT[e,q] = wq^T-contract(qT)          (fp32r matmuls, bf16 result)
#   KiT[e,k] = wk^T-contract(kT)
#   Vi[k,e]  = vT-contract(wv), plus a ones column per head (row-sum trick)
#   per head h, per k-tile: S^T[k,q] = KiT_h^T-free matmul QiT_h  (contract 64)
#     P^T = exp(S^T/8) * maskT           (no max-subtraction needed: |S|/8 <~ 20)
#     pv[e_h|rowsum, q] += Vi_aug_h^T-contract P^T
#   headiT[e,q] = pv[0:64] / pv[64]      (flash-style deferred normalization)
#   out[q,d] = headiT^T-contract wo + bias
import sys

sys.path.insert(0, "/opt/trn_rl_repo")

from contextlib import ExitStack

import numpy as np
import ml_dtypes

N, QLEN, KLEN, DMODEL, NHEAD, DK = 4, 2048, 2048, 1024, 16, 64
NCORES = 8
P = 128
QS = N * QLEN // NCORES  # 1024 q rows per core
E = NHEAD * DK  # 1024
KO = KLEN // P  # 16 k-tiles
EO = E // P  # 8 e-tiles
DO = DMODEL // P  # 8 d-tiles

_prog_cache = {}


def _build_program():
    import concourse.tile as tile
    from concourse import bacc, mybir

    f32 = mybir.dt.float32
    f32r = mybir.dt.float32r
    bf16 = mybir.dt.bfloat16
    Exp = mybir.ActivationFunctionType.Exp

    nc = bacc.Bacc("TRN2", target_bir_lowering=False, debug=False)

    qT = nc.dram_tensor("qT", (DMODEL, QS), f32r, kind="ExternalInput").ap()
    kT = nc.dram_tensor("kT", (DMODEL, KLEN), f32r, kind="ExternalInput").ap()
    vT = nc.dram_tensor("vT", (DMODEL, KLEN), f32r, kind="ExternalInput").ap()
    maskT = nc.dram_tensor("maskT", (KLEN, QS), bf16, kind="ExternalInput").ap()
    wq = nc.dram_tensor("wq", (DMODEL, E), f32r, kind="ExternalInput").ap()
    wk = nc.dram_tensor("wk", (DMODEL, E), f32r, kind="ExternalInput").ap()
    wv = nc.dram_tensor("wv", (DMODEL, E), f32r, kind="ExternalInput").ap()
    wo = nc.dram_tensor("wo", (E, DMODEL), bf16, kind="ExternalInput").ap()
    wob = nc.dram_tensor("wob", (1, DMODEL), f32, kind="ExternalInput").ap()
    out = nc.dram_tensor("out", (QS, DMODEL), f32, kind="ExternalOutput").ap()

    qT_r = qT.rearrange("(do p) q -> p do q", p=P)
    kT_r = kT.rearrange("(do p) k -> p do k", p=P)
    vT_r = vT.rearrange("(do p) k -> p do k", p=P)
    wq_r = wq.rearrange("(do p) e -> p do e", p=P)
    wk_r = wk.rearrange("(do p) e -> p do e", p=P)
    wv_r = wv.rearrange("(do p) e -> p do e", p=P)
    wo_r = wo.rearrange("(eo p) d -> p eo d", p=P)
    maskT_r = maskT.rearrange("(ko p) q -> p ko q", p=P)

    with tile.TileContext(nc) as tc, ExitStack() as top:
        res = top.enter_context(tc.tile_pool(name="res", bufs=1))
        KiT_s = res.tile([P, EO, KLEN], bf16)  # e = eo*128+p
        QiT_s = res.tile([P, EO, QS], bf16)
        Vi_s = res.tile([P, KO, NHEAD * 65], bf16)  # k = ko*128+p; col h*65+64 = 1.0
        maskT_s = res.tile([P, KO, QS], bf16)
        headiT_s = res.tile([P, EO, QS], bf16)
        wo_s = res.tile([P, EO, DMODEL], bf16)
        wob_s = res.tile([P, DMODEL], f32)

        # ---------- Phase A: KiT = (K @ WK)^T ----------
        with ExitStack() as ph:
            big = ph.enter_context(tc.tile_pool(name="ktbuf", bufs=1))
            wpool = ph.enter_context(tc.tile_pool(name="wktile", bufs=3))
            ps = ph.enter_context(tc.tile_pool(name="psA", bufs=4, space="PSUM"))
            KH = KLEN // 2
            for kh in range(2):
                kT_s = big.tile([P, DO, KH], f32r, tag="kT_s")
                for do in range(DO):
                    nc.sync.dma_start(
                        kT_s[:, do], kT_r[:, do, kh * KH : (kh + 1) * KH]
                    )
                for eo in range(EO):
                    wcol = wpool.tile([P, DO, P], f32r, tag="wcol")
                    nc.sync.dma_start(wcol[:], wk_r[:, :, eo * P : (eo + 1) * P])
                    for c in range(KH // 512):
                        pt = ps.tile([P, 512], f32)
                        for do in range(DO):
                            nc.tensor.matmul(
                                pt[:],
                                lhsT=wcol[:, do],
                                rhs=kT_s[:, do, c * 512 : (c + 1) * 512],
                                start=(do == 0),
                                stop=(do == DO - 1),
                            )
                        nc.vector.tensor_copy(
                            out=KiT_s[
                                :, eo, kh * KH + c * 512 : kh * KH + (c + 1) * 512
                            ],
                            in_=pt[:],
                        )

        # ---------- Phase B: QiT = (Q @ WQ)^T ----------
        with ExitStack() as ph:
            big = ph.enter_context(tc.tile_pool(name="qtbuf", bufs=1))
            wpool = ph.enter_context(tc.tile_pool(name="wqtile", bufs=3))
            ps = ph.enter_context(tc.tile_pool(name="psB", bufs=4, space="PSUM"))
            qT_s = big.tile([P, DO, QS], f32r)
            for do in range(DO):
                nc.sync.dma_start(qT_s[:, do], qT_r[:, do])
            for eo in range(EO):
                wcol = wpool.tile([P, DO, P], f32r, tag="wcol")
                nc.sync.dma_start(wcol[:], wq_r[:, :, eo * P : (eo + 1) * P])
                for c in range(QS // 512):
                    pt = ps.tile([P, 512], f32)
                    for do in range(DO):
                        nc.tensor.matmul(
                            pt[:],
                            lhsT=wcol[:, do],
                            rhs=qT_s[:, do, c * 512 : (c + 1) * 512],
                            start=(do == 0),
                            stop=(do == DO - 1),
                        )
                    nc.vector.tensor_copy(
                        out=QiT_s[:, eo, c * 512 : (c + 1) * 512], in_=pt[:]
                    )

        # ---------- Phase C: Vi = V @ WV (k-major) + ones columns ----------
        with ExitStack() as ph:
            wres = ph.enter_context(tc.tile_pool(name="wvres", bufs=1))
            vtp = ph.enter_context(tc.tile_pool(name="vttile", bufs=3))
            ps = ph.enter_context(tc.tile_pool(name="psC", bufs=4, space="PSUM"))
            wv_s = wres.tile([P, DO, E], f32r)
            for do in range(DO):
                nc.sync.dma_start(wv_s[:, do], wv_r[:, do])
            nc.vector.memset(Vi_s[:], 1.0)  # presets the ones columns
            for ko in range(KO):
                vcol = vtp.tile([P, DO, P], f32r, tag="vcol")
                nc.sync.dma_start(vcol[:], vT_r[:, :, ko * P : (ko + 1) * P])
                for c in range(E // 512):
                    pt = ps.tile([P, 512], f32)
                    for do in range(DO):
                        nc.tensor.matmul(
                            pt[:],
                            lhsT=vcol[:, do],
                            rhs=wv_s[:, do, c * 512 : (c + 1) * 512],
                            start=(do == 0),
                            stop=(do == DO - 1),
                        )
                    dst = Vi_s[:, ko, :].rearrange("p (h j) -> p h j", j=65)[
                        :, c * 8 : (c + 1) * 8, 0:64
                    ]
                    nc.vector.tensor_copy(
                        out=dst, in_=pt[:].rearrange("p (h j) -> p h j", j=64)
                    )

        # loads needed by phases D/E
        for ko in range(KO):
            nc.sync.dma_start(maskT_s[:, ko], maskT_r[:, ko])
        for eo in range(EO):
            nc.sync.dma_start(wo_s[:, eo], wo_r[:, eo])
        nc.sync.dma_start(wob_s[:, None, :], wob.partition_broadcast(P))

        # ---------- Phase D: attention per head ----------
        with ExitStack() as ph:
            sps = ph.enter_context(tc.tile_pool(name="spsum", bufs=2, space="PSUM"))
            pvs = ph.enter_context(tc.tile_pool(name="pvsum", bufs=2, space="PSUM"))
            pp = ph.enter_context(tc.tile_pool(name="ptile", bufs=4))
            rp = ph.enter_context(tc.tile_pool(name="recip", bufs=2))
            for h in range(NHEAD):
                eo_h, p0 = h // 2, (h % 2) * 64
                pv = pvs.tile([P, QS], f32)
                for ko in range(KO):
                    st = sps.tile([P, QS], f32)
                    for c in range(QS // 512):
                        nc.tensor.matmul(
                            st[:, c * 512 : (c + 1) * 512],
                            lhsT=KiT_s[p0 : p0 + 64, eo_h, ko * P : (ko + 1) * P],
                            rhs=QiT_s[p0 : p0 + 64, eo_h, c * 512 : (c + 1) * 512],
                            start=True,
                            stop=True,
                        )
                    ptile = pp.tile([P, QS], bf16)
                    nc.scalar.activation(out=ptile[:], in_=st[:], func=Exp, scale=0.125)
                    nc.vector.tensor_mul(
                        out=ptile[:], in0=ptile[:], in1=maskT_s[:, ko, :]
                    )
                    for c in range(QS // 512):
                        nc.tensor.matmul(
                            pv[0:65, c * 512 : (c + 1) * 512],
                            lhsT=Vi_s[:, ko, h * 65 : (h + 1) * 65],
                            rhs=ptile[:, c * 512 : (c + 1) * 512],
                            start=(ko == 0),
                            stop=(ko == KO - 1),
                            skip_group_check=True,
                        )
                # normalize: headiT_h = pv[0:64] / pv[64]
                rs = rp.tile([1, QS], f32, tag="rs")
                nc.vector.tensor_copy(out=rs[:], in_=pv[64:65, :])
                nc.vector.reciprocal(out=rs[:], in_=rs[:])
                rrep = rp.tile([64, QS], f32, tag="rrep")
                nc.gpsimd.partition_broadcast(rrep[:], rs[:])
                nc.vector.tensor_mul(
                    out=headiT_s[p0 : p0 + 64, eo_h, :], in0=pv[0:64, :], in1=rrep[:]
                )

        # ---------- Phase E: out = headiT^T @ wo + bias ----------
        with ExitStack() as ph:
            ps = ph.enter_context(tc.tile_pool(name="psE", bufs=4, space="PSUM"))
            ot = ph.enter_context(tc.tile_pool(name="otile", bufs=3))
            for qt in range(QS // P):
                for c in range(DMODEL // 512):
                    pt = ps.tile([P, 512], f32)
                    for eo in range(EO):
                        nc.tensor.matmul(
                            pt[:],
                            lhsT=headiT_s[:, eo, qt * P : (qt + 1) * P],
                            rhs=wo_s[:, eo, c * 512 : (c + 1) * 512],
                            start=(eo == 0),
                            stop=(eo == EO - 1),
                        )
                    o = ot.tile([P, 512], f32)
                    nc.vector.tensor_add(
                        out=o[:], in0=pt[:], in1=wob_s[:, c * 512 : (c + 1) * 512]
                    )
                    nc.sync.dma_start(
                        out[qt * P : (qt + 1) * P, c * 512 : (c + 1) * 512], o[:]
                    )

    nc.compile()
    return nc


def get_program():
    if "nc" not in _prog_cache:
        _prog_cache["nc"] = _build_program()
    return _prog_cache["nc"]


def make_in_maps(K, Q, V, mask, WQ, WK, WV, WO_w, WO_b):
    bf = ml_dtypes.bfloat16
    K = np.asarray(K, dtype=np.float32)
    Q = np.asarray(Q, dtype=np.float32)
    V = np.asarray(V, dtype=np.float32)
    mask = np.asarray(mask)
    # head-concat weights: (H, D, DK) -> (D, H*DK)
    wq_h = np.ascontiguousarray(
        np.asarray(WQ, dtype=np.float32).transpose(1, 0, 2).reshape(DMODEL, E)
    )
    wk_h = np.ascontiguousarray(
        np.asarray(WK, dtype=np.float32).transpose(1, 0, 2).reshape(DMODEL, E)
    )
    wv_h = np.ascontiguousarray(
        np.asarray(WV, dtype=np.float32).transpose(1, 0, 2).reshape(DMODEL, E)
    )
    wo_h = np.ascontiguousarray(np.asarray(WO_w, dtype=np.float32).T).astype(bf)
    wob_h = np.asarray(WO_b, dtype=np.float32).reshape(1, DMODEL)

    kT_b = [np.ascontiguousarray(K[n].T) for n in range(N)]
    vT_b = [np.ascontiguousarray(V[n].T) for n in range(N)]

    in_maps = []
    for c in range(NCORES):
        n, qh = c // 2, c % 2
        qs = slice(qh * QS, (qh + 1) * QS)
        in_maps.append(
            {
                "qT": np.ascontiguousarray(Q[n, qs, :].T),
                "kT": kT_b[n],
                "vT": vT_b[n],
                "maskT": np.ascontiguousarray(mask[n, 0, qs, :].T).astype(bf),
                "wq": wq_h,
                "wk": wk_h,
                "wv": wv_h,
                "wo": wo_h,
                "wob": wob_h,
            }
        )
    return in_maps


def kernel(K, Q, V, mask, WQ, WK, WV, WO_w, WO_b):
    from concourse import bass_utils

    nc = get_program()
    in_maps = make_in_maps(K, Q, V, mask, WQ, WK, WV, WO_w, WO_b)
    res = bass_utils.run_bass_kernel_spmd(
        nc, in_maps, core_ids=list(range(NCORES)), trace=False
    )
    out = np.empty((N, QLEN, DMODEL), dtype=np.float32)
    for c in range(NCORES):
        n, qh = c // 2, c % 2
        out[n, qh * QS : (qh + 1) * QS, :] = res.results[c]["out"]
    return out


# revision 10
# speedup vs baseline: 1.0471x; 1.0471x over previous
# Multi-head attention (N=4, L=2048, D=1024, H=16, DK=64) on 8 NeuronCores.
#
# Sharding: pure data-parallel over (batch n, q-half) -> 8 shards, no
# collectives. Each core gets Q rows [n, qh*1024:(qh+1)*1024], full K/V of its
# batch, and the matching mask rows.
#
# Per-core pipeline (all layouts chosen so the contraction dim sits on SBUF
# partitions and softmax row-sums come out of the PE for free):
#   QiT[e,q] = wq^T-contract(qT)          (fp32r matmuls, bf16 result)
#   KiT[e,k] = wk^T-contract(kT)
#   Vi[k,e]  = vT-contract(wv), plus a ones column per head (row-sum trick)
#   per head h, per k-tile: S^T[k,q] = KiT_h^T-free matmul QiT_h  (contract 64)
#     P^T = exp(S^T/8) * maskT           (no max-subtraction needed: |S|/8 <~ 20)
#     pv[e_h|rowsum, q] += Vi_aug_h^T-contract P^T
#   headiT[e,q] = pv[0:64] / pv[64]      (flash-style deferred normalization)
#   out[q,d] = headiT^T-contract wo + bias
import sys

sys.path.insert(0, "/opt/trn_rl_repo")

from contextlib import ExitStack

import numpy as np
import ml_dtypes

N, QLEN, KLEN, DMODEL, NHEAD, DK = 4, 2048, 2048, 1024, 16, 64
NCORES = 8
P = 128
QS = N * QLEN // NCORES  # 1024 q rows per core
E = NHEAD * DK  # 1024
KO = KLEN // P  # 16 k-tiles
EO = E // P  # 8 e-tiles
DO = DMODEL // P  # 8 d-tiles

_prog_cache = {}


def _build_program():
    import concourse.tile as tile
    from concourse import bacc, mybir

    f32 = mybir.dt.float32
    f32r = mybir.dt.float32r
    bf16 = mybir.dt.bfloat16
    Exp = mybir.ActivationFunctionType.Exp

    nc = bacc.Bacc("TRN2", target_bir_lowering=False, debug=False)

    qT = nc.dram_tensor("qT", (DMODEL, QS), f32r, kind="ExternalInput").ap()
    kT = nc.dram_tensor("kT", (DMODEL, KLEN), f32r, kind="ExternalInput").ap()
    vT = nc.dram_tensor("vT", (DMODEL, KLEN), f32r, kind="ExternalInput").ap()
    maskT = nc.dram_tensor("maskT", (KLEN, QS), bf16, kind="ExternalInput").ap()
    wq = nc.dram_tensor("wq", (DMODEL, E), f32r, kind="ExternalInput").ap()
    wk = nc.dram_tensor("wk", (DMODEL, E), f32r, kind="ExternalInput").ap()
    wv = nc.dram_tensor("wv", (DMODEL, E), f32r, kind="ExternalInput").ap()
    wo = nc.dram_tensor("wo", (E, DMODEL), bf16, kind="ExternalInput").ap()
    wob = nc.dram_tensor("wob", (1, DMODEL), f32, kind="ExternalInput").ap()
    out = nc.dram_tensor("out", (QS, DMODEL), f32, kind="ExternalOutput").ap()

    qT_r = qT.rearrange("(do p) q -> p do q", p=P)
    kT_r = kT.rearrange("(do p) k -> p do k", p=P)
    vT_r = vT.rearrange("(do p) k -> p do k", p=P)
    wq_r = wq.rearrange("(do p) e -> p do e", p=P)
    wk_r = wk.rearrange("(do p) e -> p do e", p=P)
    wv_r = wv.rearrange("(do p) e -> p do e", p=P)
    wo_r = wo.rearrange("(eo p) d -> p eo d", p=P)
    maskT_r = maskT.rearrange("(ko p) q -> p ko q", p=P)

    with tile.TileContext(nc) as tc, ExitStack() as top:
        res = top.enter_context(tc.tile_pool(name="res", bufs=1))
        KiT_s = res.tile([P, EO, KLEN], bf16)  # e = eo*128+p
        QiT_s = res.tile([P, EO, QS], bf16)
        Vi_s = res.tile([P, KO, NHEAD * 65], bf16)  # k = ko*128+p; col h*65+64 = 1.0
        maskT_s = res.tile([P, KO, QS], bf16)
        headiT_s = res.tile([P, EO, QS], bf16)
        wo_s = res.tile([P, EO, DMODEL], bf16)
        wob_s = res.tile([P, DMODEL], f32)

        # ---------- Phase A: KiT = (K @ WK)^T ----------
        with ExitStack() as ph:
            big = ph.enter_context(tc.tile_pool(name="ktbuf", bufs=1))
            wpool = ph.enter_context(tc.tile_pool(name="wktile", bufs=3))
            ps = ph.enter_context(tc.tile_pool(name="psA", bufs=4, space="PSUM"))
            KH = KLEN // 2
            for kh in range(2):
                kT_s = big.tile([P, DO, KH], f32r, tag="kT_s")
                for do in range(DO):
                    nc.sync.dma_start(
                        kT_s[:, do], kT_r[:, do, kh * KH : (kh + 1) * KH]
                    )
                for eo in range(EO):
                    wcol = wpool.tile([P, DO, P], f32r, tag="wcol")
                    nc.sync.dma_start(wcol[:], wk_r[:, :, eo * P : (eo + 1) * P])
                    for c in range(KH // 512):
                        pt = ps.tile([P, 512], f32)
                        for do in range(DO):
                            nc.tensor.matmul(
                                pt[:],
                                lhsT=wcol[:, do],
                                rhs=kT_s[:, do, c * 512 : (c + 1) * 512],
                                start=(do == 0),
                                stop=(do == DO - 1),
                            )
                        nc.vector.tensor_copy(
                            out=KiT_s[
                                :, eo, kh * KH + c * 512 : kh * KH + (c + 1) * 512
                            ],
                            in_=pt[:],
                        )

        # ---------- Phase B: QiT = (Q @ WQ)^T ----------
        with ExitStack() as ph:
            big = ph.enter_context(tc.tile_pool(name="qtbuf", bufs=1))
            wpool = ph.enter_context(tc.tile_pool(name="wqtile", bufs=3))
            ps = ph.enter_context(tc.tile_pool(name="psB", bufs=4, space="PSUM"))
            qT_s = big.tile([P, DO, QS], f32r)
            for do in range(DO):
                nc.sync.dma_start(qT_s[:, do], qT_r[:, do])
            for eo in range(EO):
                wcol = wpool.tile([P, DO, P], f32r, tag="wcol")
                nc.sync.dma_start(wcol[:], wq_r[:, :, eo * P : (eo + 1) * P])
                for c in range(QS // 512):
                    pt = ps.tile([P, 512], f32)
                    for do in range(DO):
                        nc.tensor.matmul(
                            pt[:],
                            lhsT=wcol[:, do],
                            rhs=qT_s[:, do, c * 512 : (c + 1) * 512],
                            start=(do == 0),
                            stop=(do == DO - 1),
                        )
                    nc.vector.tensor_copy(
                        out=QiT_s[:, eo, c * 512 : (c + 1) * 512], in_=pt[:]
                    )

        # ---------- Phase C: Vi = V @ WV (k-major) + ones columns ----------
        with ExitStack() as ph:
            wres = ph.enter_context(tc.tile_pool(name="wvres", bufs=1))
            vtp = ph.enter_context(tc.tile_pool(name="vttile", bufs=3))
            ps = ph.enter_context(tc.tile_pool(name="psC", bufs=4, space="PSUM"))
            wv_s = wres.tile([P, DO, E], f32r)
            for do in range(DO):
                nc.sync.dma_start(wv_s[:, do], wv_r[:, do])
            nc.vector.memset(Vi_s[:, :, 64::65], 1.0)  # the ones columns
            for ko in range(KO):
                vcol = vtp.tile([P, DO, P], f32r, tag="vcol")
                nc.sync.dma_start(vcol[:], vT_r[:, :, ko * P : (ko + 1) * P])
                for c in range(E // 512):
                    pt = ps.tile([P, 512], f32)
                    for do in range(DO):
                        nc.tensor.matmul(
                            pt[:],
                            lhsT=vcol[:, do],
                            rhs=wv_s[:, do, c * 512 : (c + 1) * 512],
                            start=(do == 0),
                            stop=(do == DO - 1),
                        )
                    dst = Vi_s[:, ko, :].rearrange("p (h j) -> p h j", j=65)[
                        :, c * 8 : (c + 1) * 8, 0:64
                    ]
                    nc.vector.tensor_copy(
                        out=dst, in_=pt[:].rearrange("p (h j) -> p h j", j=64)
                    )

        # loads needed by phases D/E
        for ko in range(KO):
            nc.sync.dma_start(maskT_s[:, ko], maskT_r[:, ko])
        for eo in range(EO):
            nc.sync.dma_start(wo_s[:, eo], wo_r[:, eo])
        nc.sync.dma_start(wob_s[:, None, :], wob.partition_broadcast(P))

        # ---------- Phase D: attention, two heads interleaved per pair ----------
        # Heads 2hp (partitions 0-63) and 2hp+1 (64-127) alternate so their
        # K=64 S^T matmuls land in different PE row groups and overlap; PV for
        # k-tile ko-1 is emitted after S^T(ko) to hide the exp+mask latency.
        rs_all = res.tile([NHEAD, QS], f32)
        with ExitStack() as ph:
            sps = ph.enter_context(tc.tile_pool(name="spsum", bufs=1, space="PSUM"))
            pvs = ph.enter_context(tc.tile_pool(name="pvsum", bufs=1, space="PSUM"))
            pp = ph.enter_context(tc.tile_pool(name="ptile", bufs=2))
            for hp in range(NHEAD // 2):
                pv = [
                    pvs.tile([P, QS], f32, tag=f"pv{i}", name=f"pv{i}_{hp}")
                    for i in range(2)
                ]
                pts = [None, None]

                def emit_pv(ko, hp=hp, pv=pv, pts=pts):
                    for i in range(2):
                        h = 2 * hp + i
                        for c in range(QS // 512):
                            nc.tensor.matmul(
                                pv[i][0:65, c * 512 : (c + 1) * 512],
                                lhsT=Vi_s[:, ko, h * 65 : (h + 1) * 65],
                                rhs=pts[i][:, c * 512 : (c + 1) * 512],
                                start=(ko == 0),
                                stop=(ko == KO - 1),
                                skip_group_check=True,
                            )

                for ko in range(KO):
                    newpts = [None, None]
                    for i in range(2):
                        h, p0 = 2 * hp + i, 64 * i
                        st = sps.tile([P, QS], f32, tag=f"st{i}")
                        for c in range(QS // 512):
                            nc.tensor.matmul(
                                st[:, c * 512 : (c + 1) * 512],
                                lhsT=KiT_s[p0 : p0 + 64, hp, ko * P : (ko + 1) * P],
                                rhs=QiT_s[p0 : p0 + 64, hp, c * 512 : (c + 1) * 512],
                                start=True,
                                stop=True,
                            )
                        pt = pp.tile([P, QS], bf16, tag=f"pt{i}")
                        nc.scalar.activation(
                            out=pt[:], in_=st[:], func=Exp, scale=0.125
                        )
                        nc.vector.tensor_mul(
                            out=pt[:], in0=pt[:], in1=maskT_s[:, ko, :]
                        )
                        newpts[i] = pt
                    if ko >= 1:
                        emit_pv(ko - 1)
                    pts[0], pts[1] = newpts
                emit_pv(KO - 1)
                # copy out unnormalized heads + row sums (normalized later)
                for i in range(2):
                    h = 2 * hp + i
                    nc.vector.tensor_copy(
                        out=headiT_s[64 * i : 64 * i + 64, hp, :], in_=pv[i][0:64, :]
                    )
                    rstmp = pp.tile([1, QS], f32, tag="rstmp", name=f"rstmp_{h}")
                    nc.vector.tensor_copy(out=rstmp[:], in_=pv[i][64:65, :])
                    nc.sync.dma_start(rs_all[h : h + 1, :], rstmp[:])

        # deferred normalization: one packed reciprocal, DMA-broadcast via DRAM
        rs_dram = nc.dram_tensor("rs_scratch", (NHEAD, QS), f32).ap()
        with ExitStack() as ph:
            rp = ph.enter_context(tc.tile_pool(name="recip", bufs=2))
            rsinv = rp.tile([NHEAD, QS], f32, tag="rsinv")
            nc.vector.reciprocal(out=rsinv[:], in_=rs_all[:])
            nc.sync.dma_start(rs_dram[:, :], rsinv[:])
            for hp in range(NHEAD // 2):
                rrep = rp.tile([P, QS], f32, tag="rrep", name=f"rrep_{hp}")
                nc.sync.dma_start(
                    rrep[0:64, None, :],
                    rs_dram[2 * hp : 2 * hp + 1, :].partition_broadcast(64),
                )
                nc.sync.dma_start(
                    rrep[64:128, None, :],
                    rs_dram[2 * hp + 1 : 2 * hp + 2, :].partition_broadcast(64),
                )
                nc.vector.tensor_mul(
                    out=headiT_s[:, hp, :], in0=headiT_s[:, hp, :], in1=rrep[:]
                )

        # ---------- Phase E: out = headiT^T @ wo + bias ----------
        with ExitStack() as ph:
            ps = ph.enter_context(tc.tile_pool(name="psE", bufs=4, space="PSUM"))
            ot = ph.enter_context(tc.tile_pool(name="otile", bufs=3))
            for qt in range(QS // P):
                for c in range(DMODEL // 512):
                    pt = ps.tile([P, 512], f32)
                    for eo in range(EO):
                        nc.tensor.matmul(
                            pt[:],
                            lhsT=headiT_s[:, eo, qt * P : (qt + 1) * P],
                            rhs=wo_s[:, eo, c * 512 : (c + 1) * 512],
                            start=(eo == 0),
                            stop=(eo == EO - 1),
                        )
                    o = ot.tile([P, 512], f32)
                    nc.vector.tensor_add(
                        out=o[:], in0=pt[:], in1=wob_s[:, c * 512 : (c + 1) * 512]
                    )
                    nc.sync.dma_start(
                        out[qt * P : (qt + 1) * P, c * 512 : (c + 1) * 512], o[:]
                    )

    nc.compile()
    return nc


def get_program():
    if "nc" not in _prog_cache:
        _prog_cache["nc"] = _build_program()
    return _prog_cache["nc"]


def make_in_maps(K, Q, V, mask, WQ, WK, WV, WO_w, WO_b):
    bf = ml_dtypes.bfloat16
    K = np.asarray(K, dtype=np.float32)
    Q = np.asarray(Q, dtype=np.float32)
    V = np.asarray(V, dtype=np.float32)
    mask = np.asarray(mask)
    # head-concat weights: (H, D, DK) -> (D, H*DK)
    wq_h = np.ascontiguousarray(
        np.asarray(WQ, dtype=np.float32).transpose(1, 0, 2).reshape(DMODEL, E)
    )
    wk_h = np.ascontiguousarray(
        np.asarray(WK, dtype=np.float32).transpose(1, 0, 2).reshape(DMODEL, E)
    )
    wv_h = np.ascontiguousarray(
        np.asarray(WV, dtype=np.float32).transpose(1, 0, 2).reshape(DMODEL, E)
    )
    wo_h = np.ascontiguousarray(np.asarray(WO_w, dtype=np.float32).T).astype(bf)
    wob_h = np.asarray(WO_b, dtype=np.float32).reshape(1, DMODEL)

    kT_b = [np.ascontiguousarray(K[n].T) for n in range(N)]
    vT_b = [np.ascontiguousarray(V[n].T) for n in range(N)]

    in_maps = []
    for c in range(NCORES):
        n, qh = c // 2, c % 2
        qs = slice(qh * QS, (qh + 1) * QS)
        in_maps.append(
            {
                "qT": np.ascontiguousarray(Q[n, qs, :].T),
                "kT": kT_b[n],
                "vT": vT_b[n],
                "maskT": np.ascontiguousarray(mask[n, 0, qs, :].T).astype(bf),
                "wq": wq_h,
                "wk": wk_h,
                "wv": wv_h,
                "wo": wo_h,
                "wob": wob_h,
            }
        )
    return in_maps


def kernel(K, Q, V, mask, WQ, WK, WV, WO_w, WO_b):
    from concourse import bass_utils

    nc = get_program()
    in_maps = make_in_maps(K, Q, V, mask, WQ, WK, WV, WO_w, WO_b)
    res = bass_utils.run_bass_kernel_spmd(
        nc, in_maps, core_ids=list(range(NCORES)), trace=False
    )
    out = np.empty((N, QLEN, DMODEL), dtype=np.float32)
    for c in range(NCORES):
        n, qh = c // 2, c % 2
        out[n, qh * QS : (qh + 1) * QS, :] = res.results[c]["out"]
    return out
